# revision 3
# baseline (speedup 1.0000x reference)
"""Trainium2 Bass kernel for CPELite (gnn_message_passing).

Data-parallel over N=1M points on 8 NeuronCores. Per core:
  phase H: partial segment-sum of (coord, 1) over 4096 patches via
           bf16 one-hot radix (hi=128 x lo=32) matmuls into PSUM.
  AllReduce of the [4096,4] table, centers = sums/max(cnt,1).
  phase M: per 128-pt tile: indirect-DMA gather centers[pid],
           delta = coord-center, PE transpose to feature-major,
           h = gelu(W1^T deltaT + b1), out2T = W2p^T h + Wp^T xT,
           transpose-back with fused bias/row-sum, LayerNorm, store.

Weight folding (host): Wp = diag(dw_w) @ pw_W ; W2p = W2 @ Wp ;
 bpp = b2 @ Wp + dw_b @ pw_W + pw_b. So out2 = (x+pos)*dw_w+dw_b) @ pw_W + pw_b
 == x @ Wp + gelu(...) @ W2p + bpp.
"""

import math

import ml_dtypes
import numpy as np

import concourse.bacc as bacc
import concourse.bass as bass
import concourse.mybir as mybir
import concourse.tile as tile
from concourse.bass_utils import run_bass_kernel_spmd
from concourse.masks import make_identity

N_CORES = 8
DIM = 64
HID = 64
P = 4096
EPS = 1e-5

F32 = mybir.dt.float32
BF16 = mybir.dt.bfloat16
I32 = mybir.dt.int32

_PROGRAM_CACHE = {}
_NO_GATHER = False


def _build(NP, fast_ln, repeat=1, collective=True):
    """Build the per-core SPMD program. NP = padded points per core.

    repeat>1 re-runs the main phase (identical output) for wall-clock
    differencing in test.py. collective=False (sim-only) replaces the
    AllReduce with a local copy so single-core CoreSim can run."""
    assert NP % 128 == 0
    n_tiles = NP // 128
    ST = 16  # point-tiles per supertile
    n_full = n_tiles // ST
    tail = n_tiles - n_full * ST
    st_sizes = [ST] * n_full + ([tail] if tail else [])

    nc = bacc.Bacc("TRN2", num_swdge_queues=4)

    x_in = nc.dram_tensor("x", [NP, DIM], F32, kind="ExternalInput")
    coord_in = nc.dram_tensor("coord", [NP, 3], F32, kind="ExternalInput")
    pidg_in = nc.dram_tensor("pidg", [NP], I32, kind="ExternalInput")
    pidhi_in = nc.dram_tensor("pidhi", [NP], BF16, kind="ExternalInput")
    pidlo_in = nc.dram_tensor("pidlo", [NP], BF16, kind="ExternalInput")
    w1_in = nc.dram_tensor("w1", [3, HID], F32, kind="ExternalInput")
    b1_in = nc.dram_tensor("b1", [HID, 1], F32, kind="ExternalInput")
    w2p_in = nc.dram_tensor("w2p", [HID, DIM], F32, kind="ExternalInput")
    wp_in = nc.dram_tensor("wp", [DIM, DIM], F32, kind="ExternalInput")
    rhs65_in = nc.dram_tensor("rhs65", [DIM + 1, DIM + 1], F32, kind="ExternalInput")
    chi_in = nc.dram_tensor("chi", [128, 128], BF16, kind="ExternalInput")
    clo_in = nc.dram_tensor("clo", [128, 32], BF16, kind="ExternalInput")
    grep_in = nc.dram_tensor("grep", [128, DIM], F32, kind="ExternalInput")
    brep_in = nc.dram_tensor("brep", [128, DIM], F32, kind="ExternalInput")
    y_out = nc.dram_tensor("y", [NP, DIM], F32, kind="ExternalOutput")

    with tile.TileContext(nc) as tc:
        with (
            tc.tile_pool(name="const", bufs=1) as cpool,
            tc.tile_pool(name="dram", bufs=1, space="DRAM") as dpool,
        ):
            # constants
            w1pad = cpool.tile([67, HID], F32)
            nc.sync.dma_start(out=w1pad[64:67, :], in_=w1_in[:])
            w1_sb = w1pad[64:67, :]
            b1_sb = cpool.tile([HID, 1], F32)
            nc.sync.dma_start(out=b1_sb[:], in_=b1_in[:])
            w2p_sb = cpool.tile([HID, DIM], F32)
            nc.sync.dma_start(out=w2p_sb[:], in_=w2p_in[:])
            wp_sb = cpool.tile([DIM, DIM], F32)
            nc.sync.dma_start(out=wp_sb[:], in_=wp_in[:])
            rhs65_sb = cpool.tile([DIM + 1, DIM + 1], F32)
            nc.sync.dma_start(out=rhs65_sb[:], in_=rhs65_in[:])
            chi_sb = cpool.tile([128, 128], BF16)
            nc.sync.dma_start(out=chi_sb[:], in_=chi_in[:])
            clo_sb = cpool.tile([128, 32], BF16)
            nc.sync.dma_start(out=clo_sb[:], in_=clo_in[:])
            ident_sb = cpool.tile([128, 128], F32)
            make_identity(nc, ident_sb[:])
            eps_sb = cpool.tile([128, 1], F32)
            nc.vector.memset(eps_sb[:], EPS)
            if not fast_ln:
                grep_sb = cpool.tile([128, DIM], F32)
                nc.sync.dma_start(out=grep_sb[:], in_=grep_in[:])
                brep_sb = cpool.tile([128, DIM], F32)
                nc.sync.dma_start(out=brep_sb[:], in_=brep_in[:])

            hist_dram = dpool.tile([P, 4], F32)
            hist_ar = dpool.tile([P, 4], F32)
            centers_dram = dpool.tile([P, 4], F32)
            centers_pad = dpool.tile([P, 64], F32)

            # ---------------- phase H: histogram ----------------
            with (
                tc.tile_pool(name="hsb", bufs=3) as hpool,
                tc.tile_pool(name="hwork", bufs=2) as wpool,
                tc.tile_pool(name="hps", bufs=1, space="PSUM") as hps,
            ):
                psum_hist = hps.tile([128, 128], F32, space="PSUM")
                gtile = 0
                t0 = 0
                for nt in st_sizes:
                    npts = nt * 128
                    phi = hpool.tile([128, ST], BF16, tag="phi")
                    nc.sync.dma_start(
                        out=phi[:, :nt],
                        in_=pidhi_in[t0 : t0 + npts].rearrange("(j p) -> p j", p=128),
                    )
                    plo = hpool.tile([128, ST], BF16, tag="plo")
                    nc.sync.dma_start(
                        out=plo[:, :nt],
                        in_=pidlo_in[t0 : t0 + npts].rearrange("(j p) -> p j", p=128),
                    )
                    c4 = hpool.tile([128, ST, 4], F32, tag="c4")
                    nc.vector.memset(c4[:, :nt, 3], 1.0)
                    nc.sync.dma_start(
                        out=c4[:, :nt, 0:3],
                        in_=coord_in[t0 : t0 + npts, :].rearrange(
                            "(j p) c -> p j c", p=128
                        ),
                    )
                    oh_hi = wpool.tile([128, ST, 128], BF16, tag="ohhi")
                    nc.vector.tensor_tensor(
                        out=oh_hi[:, :nt],
                        in0=phi[:, :nt, None].to_broadcast([128, nt, 128]),
                        in1=chi_sb[:, None, :].to_broadcast([128, nt, 128]),
                        op=mybir.AluOpType.is_equal,
                    )
                    oh_lo = wpool.tile([128, ST, 32], BF16, tag="ohlo")
                    nc.vector.tensor_tensor(
                        out=oh_lo[:, :nt],
                        in0=plo[:, :nt, None].to_broadcast([128, nt, 32]),
                        in1=clo_sb[:, None, :].to_broadcast([128, nt, 32]),
                        op=mybir.AluOpType.is_equal,
                    )
                    r_all = wpool.tile([128, ST, 4, 32], BF16, tag="rall")
                    nc.vector.tensor_tensor(
                        out=r_all[:, :nt],
                        in0=c4[:, :nt, :, None].to_broadcast([128, nt, 4, 32]),
                        in1=oh_lo[:, :nt, None, :].to_broadcast([128, nt, 4, 32]),
                        op=mybir.AluOpType.mult,
                    )
                    for j in range(nt):
                        nc.tensor.matmul(
                            out=psum_hist[:],
                            lhsT=r_all[:, j].rearrange("p a b -> p (a b)"),
                            rhs=oh_hi[:, j],
                            start=(gtile == 0),
                            stop=(gtile == n_tiles - 1),
                        )
                        gtile += 1
                    t0 += npts
                hist_sb = wpool.tile([128, 128], F32, tag="hsb")
                nc.vector.tensor_copy(out=hist_sb[:], in_=psum_hist[:])
                # hist_sb[(d,lo), hi] -> hist_dram[(hi,lo), d]
                for d in range(4):
                    nc.sync.dma_start(
                        out=hist_dram[:]
                        .rearrange("(h l) d -> l h d", l=32)[:, :, d]
                        .rearrange("l h -> l h"),
                        in_=hist_sb[32 * d : 32 * d + 32, :],
                    )

            # all-reduce + centers
            with tc.tile_pool(name="cwork", bufs=1) as cw:
                if collective:
                    nc.gpsimd.collective_compute(
                        "AllReduce",
                        mybir.AluOpType.add,
                        replica_groups=[list(range(N_CORES))],
                        ins=[hist_dram.opt()],
                        outs=[hist_ar.opt()],
                    )
                else:
                    nc.sync.dma_start(out=hist_ar[:], in_=hist_dram[:])
                htab = cw.tile([128, 32, 4], F32)
                nc.sync.dma_start(
                    out=htab[:],
                    in_=hist_ar[:].rearrange("(h l) d -> h (l d)", l=32),
                )
                cnt = cw.tile([128, 32], F32)
                nc.vector.tensor_scalar(
                    out=cnt[:],
                    in0=htab[:, :, 3],
                    scalar1=1.0,
                    scalar2=None,
                    op0=mybir.AluOpType.max,
                )
                rec = cw.tile([128, 32], F32)
                nc.vector.reciprocal(out=rec[:], in_=cnt[:])
                ctab = cw.tile([128, 32, 4], F32)
                nc.vector.tensor_tensor(
                    out=ctab[:],
                    in0=htab[:],
                    in1=rec[:, :, None].to_broadcast([128, 32, 4]),
                    op=mybir.AluOpType.mult,
                )
                nc.sync.dma_start(
                    out=centers_dram[:].rearrange("(h l) d -> h (l d)", l=32),
                    in_=ctab[:],
                )
                nc.sync.dma_start(
                    out=centers_pad[:].rearrange("(h l) d -> h l d", l=32)[:, :, 0:4],
                    in_=ctab[:],
                )

            # ---------------- phase M: main ----------------
            with (
                tc.tile_pool(name="min", bufs=3) as mpool,
                tc.tile_pool(name="mmid", bufs=2) as mid,
                tc.tile_pool(name="mout", bufs=2) as mo,
                tc.tile_pool(name="mps", bufs=2, space="PSUM") as mps,
            ):
                n_st = len(st_sizes)
                cg_all = mpool.tile([128, n_st, ST, 4], F32, tag="cgall")
                for _rep in range(repeat):
                  t0 = 0
                  for sti, nt in enumerate(st_sizes):
                    npts = nt * 128
                    W = nt * 128  # wide free dim

                    xc = mpool.tile([128, ST, 68], F32, tag="xc")
                    nc.sync.dma_start(
                        out=xc[:, :nt, 0:64],
                        in_=x_in[t0 : t0 + npts, :].rearrange(
                            "(j p) f -> p j f", p=128
                        ),
                    )
                    nc.sync.dma_start(
                        out=xc[:, :nt, 64:67],
                        in_=coord_in[t0 : t0 + npts, :].rearrange(
                            "(j p) c -> p j c", p=128
                        ),
                    )
                    pidg = mpool.tile([128, ST], I32, tag="pidg")
                    nc.sync.dma_start(
                        out=pidg[:, :nt],
                        in_=pidg_in[t0 : t0 + npts].rearrange("(j p) -> p j", p=128),
                    )
                    # gather centers + delta (per-tile indirect DMAs, prefetched)
                    if not _NO_GATHER:
                        cg = cg_all[:, sti]
                        for j in range(nt):
                            nc.gpsimd.indirect_dma_start(
                                out=cg[:, j],
                                out_offset=None,
                                in_=centers_dram[:],
                                in_offset=bass.IndirectOffsetOnAxis(
                                    ap=pidg[:, j : j + 1], axis=0
                                ),
                            )
                        nc.vector.tensor_tensor(
                            out=xc[:, :nt, 64:67],
                            in0=xc[:, :nt, 64:67],
                            in1=cg[:, :nt, 0:3],
                            op=mybir.AluOpType.subtract,
                        )
                    # transpose in
                    ps_in = mps.tile([68, ST * 128], F32, space="PSUM", tag="ps")
                    for j in range(nt):
                        nc.tensor.transpose(
                            out=ps_in[:, j * 128 : (j + 1) * 128],
                            in_=xc[:, j, 0:68],
                            identity=ident_sb[:],
                        )
                    xcT = mid.tile([68, ST * 128], F32, tag="xcT")
                    nc.vector.tensor_copy(out=xcT[:, :W], in_=ps_in[:, :W])
                    # mlp1
                    ps_h = mps.tile([HID, ST * 128], F32, space="PSUM", tag="ps")
                    for q in range(0, W, 512):
                        qe = min(q + 512, W)
                        nc.tensor.matmul(
                            out=ps_h[:, q:qe],
                            lhsT=w1_sb,
                            rhs=xcT[64:67, q:qe],
                            start=True,
                            stop=True,
                        )
                    g_sb = mid.tile([HID, ST * 128], F32, tag="gsb")
                    nc.scalar.activation(
                        out=g_sb[:, :W],
                        in_=ps_h[:, :W],
                        func=mybir.ActivationFunctionType.Gelu,
                        bias=b1_sb[:],
                        scale=1.0,
                    )
                    # mlp2 + pw
                    ps_o = mps.tile([DIM, ST * 128], F32, space="PSUM", tag="ps")
                    for q in range(0, W, 512):
                        qe = min(q + 512, W)
                        nc.tensor.matmul(
                            out=ps_o[:, q:qe],
                            lhsT=w2p_sb[:],
                            rhs=g_sb[:, q:qe],
                            start=True,
                            stop=False,
                        )
                    for q in range(0, W, 512):
                        qe = min(q + 512, W)
                        nc.tensor.matmul(
                            out=ps_o[:, q:qe],
                            lhsT=wp_sb[:],
                            rhs=xcT[0:64, q:qe],
                            start=False,
                            stop=True,
                        )
                    out2 = mid.tile([DIM + 1, ST * 128], F32, tag="out2")
                    nc.vector.tensor_copy(out=out2[0:64, :W], in_=ps_o[:, :W])
                    nc.gpsimd.memset(out2[64:65, :W], 1.0)
                    # transpose out (fused bias & row-sum via rhs65)
                    ps_y = mps.tile([128, ST, 128], F32, space="PSUM", tag="ps")
                    for j in range(nt):
                        nc.tensor.matmul(
                            out=ps_y[:, j, 0:65],
                            lhsT=out2[:, j * 128 : (j + 1) * 128],
                            rhs=rhs65_sb[:],
                            start=True,
                            stop=True,
                        )
                    w_view = ps_y[:, :nt, 0:64]
                    s_view = ps_y[:, :nt, 64]
                    mu = mo.tile([128, ST], F32, tag="mu")
                    nc.vector.tensor_scalar(
                        out=mu[:, :nt],
                        in0=s_view,
                        scalar1=1.0 / DIM,
                        scalar2=None,
                        op0=mybir.AluOpType.mult,
                    )
                    scr = mo.tile([128, ST, DIM], F32, tag="scr")
                    nc.scalar.activation(
                        out=scr[:, :nt],
                        in_=w_view,
                        func=mybir.ActivationFunctionType.Square,
                    )
                    ssq = mo.tile([128, ST], F32, tag="ssq")
                    nc.vector.tensor_reduce(
                        out=ssq[:, :nt],
                        in_=scr[:, :nt],
                        axis=mybir.AxisListType.X,
                        op=mybir.AluOpType.add,
                    )
                    var = mo.tile([128, ST], F32, tag="var")
                    # var = ssq/64 - mu^2
                    nc.vector.tensor_scalar(
                        out=ssq[:, :nt],
                        in0=ssq[:, :nt],
                        scalar1=1.0 / DIM,
                        scalar2=None,
                        op0=mybir.AluOpType.mult,
                    )
                    nc.vector.tensor_tensor(
                        out=var[:, :nt],
                        in0=mu[:, :nt],
                        in1=mu[:, :nt],
                        op=mybir.AluOpType.mult,
                    )
                    nc.vector.tensor_tensor(
                        out=var[:, :nt],
                        in0=ssq[:, :nt],
                        in1=var[:, :nt],
                        op=mybir.AluOpType.subtract,
                    )
                    sd = mo.tile([128, ST], F32, tag="sd")
                    nc.scalar.activation(
                        out=sd[:, :nt],
                        in_=var[:, :nt],
                        func=mybir.ActivationFunctionType.Sqrt,
                        bias=eps_sb[:],
                    )
                    inv = mo.tile([128, ST], F32, tag="inv")
                    nc.vector.reciprocal(out=inv[:, :nt], in_=sd[:, :nt])
                    t1 = mo.tile([128, ST, DIM], F32, tag="t1")
                    nc.vector.tensor_tensor(
                        out=t1[:, :nt],
                        in0=w_view,
                        in1=mu[:, :nt, None].to_broadcast([128, nt, DIM]),
                        op=mybir.AluOpType.subtract,
                    )
                    y_sb = mo.tile([128, ST, DIM], F32, tag="ysb")
                    nc.vector.tensor_tensor(
                        out=y_sb[:, :nt],
                        in0=t1[:, :nt],
                        in1=inv[:, :nt, None].to_broadcast([128, nt, DIM]),
                        op=mybir.AluOpType.mult,
                    )
                    if not fast_ln:
                        nc.vector.tensor_tensor(
                            out=y_sb[:, :nt],
                            in0=y_sb[:, :nt],
                            in1=grep_sb[:, None, :].to_broadcast([128, nt, DIM]),
                            op=mybir.AluOpType.mult,
                        )
                        nc.vector.tensor_tensor(
                            out=y_sb[:, :nt],
                            in0=y_sb[:, :nt],
                            in1=brep_sb[:, None, :].to_broadcast([128, nt, DIM]),
                            op=mybir.AluOpType.add,
                        )
                    nc.sync.dma_start(
                        out=y_out[t0 : t0 + npts, :].rearrange(
                            "(j p) f -> p j f", p=128
                        ),
                        in_=y_sb[:, :nt],
                    )
                    t0 += npts

    nc.finalize()
    return nc


def _prep(inputs, NP):
    """Host-side prep: fold weights, build per-core input maps."""
    x = np.ascontiguousarray(np.asarray(inputs["x"], dtype=np.float32))
    coord = np.ascontiguousarray(np.asarray(inputs["coord"], dtype=np.float32))
    pid = np.asarray(inputs["patch_ids"]).astype(np.int64)
    N = x.shape[0]

    W1 = np.asarray(inputs["W1"], dtype=np.float64)
    b1 = np.asarray(inputs["b1"], dtype=np.float64)
    W2 = np.asarray(inputs["W2"], dtype=np.float64)
    b2 = np.asarray(inputs["b2"], dtype=np.float64)
    dw_w = np.asarray(inputs["dw_w"], dtype=np.float64)
    dw_b = np.asarray(inputs["dw_b"], dtype=np.float64)
    pw_W = np.asarray(inputs["pw_W"], dtype=np.float64)
    pw_b = np.asarray(inputs["pw_b"], dtype=np.float64)
    ln_g = np.asarray(inputs["ln_g"], dtype=np.float64)
    ln_b = np.asarray(inputs["ln_b"], dtype=np.float64)

    Wp = dw_w[:, None] * pw_W
    W2p = W2 @ Wp
    bpp = b2 @ Wp + dw_b @ pw_W + pw_b

    fast_ln = bool(np.all(ln_g == 1.0) and np.all(ln_b == 0.0))

    rhs65 = np.zeros((DIM + 1, DIM + 1), dtype=np.float64)
    rhs65[:DIM, :DIM] = np.eye(DIM)
    rhs65[DIM, :DIM] = bpp
    rhs65[:DIM, DIM] = 1.0
    rhs65[DIM, DIM] = bpp.sum()

    chi = np.broadcast_to(np.arange(128, dtype=np.float32), (128, 128))
    clo = np.broadcast_to(np.arange(32, dtype=np.float32), (128, 32))

    common = dict(
        w1=np.asarray(W1, dtype=np.float32),
        b1=np.asarray(b1, dtype=np.float32).reshape(HID, 1),
        w2p=np.asarray(W2p, dtype=np.float32),
        wp=np.asarray(Wp, dtype=np.float32),
        rhs65=np.asarray(rhs65, dtype=np.float32),
        chi=chi.astype(ml_dtypes.bfloat16),
        clo=clo.astype(ml_dtypes.bfloat16),
        grep=np.broadcast_to(ln_g.astype(np.float32), (128, DIM)).copy(),
        brep=np.broadcast_to(ln_b.astype(np.float32), (128, DIM)).copy(),
    )

    pid_hi = (pid >> 5).astype(np.float32)
    pid_lo = (pid & 31).astype(np.float32)
    pid_g = pid.astype(np.int32)

    in_maps = []
    for c in range(N_CORES):
        a, b = c * NP, min((c + 1) * NP, N)
        pad = NP - (b - a)
        if pad == 0:
            xs, cs = x[a:b], coord[a:b]
            hi_s, lo_s, pg = pid_hi[a:b], pid_lo[a:b], pid_g[a:b]
        else:
            xs = np.concatenate([x[a:b], np.zeros((pad, DIM), np.float32)])
            cs = np.concatenate([coord[a:b], np.zeros((pad, 3), np.float32)])
            hi_s = np.concatenate([pid_hi[a:b], np.full(pad, 1000.0, np.float32)])
            lo_s = np.concatenate([pid_lo[a:b], np.full(pad, 1000.0, np.float32)])
            pg = np.concatenate([pid_g[a:b], np.zeros(pad, np.int32)])
        m = dict(common)
        m.update(
            x=xs,
            coord=cs,
            pidg=pg,
            pidhi=hi_s.astype(ml_dtypes.bfloat16),
            pidlo=lo_s.astype(ml_dtypes.bfloat16),
        )
        in_maps.append(m)
    return in_maps, fast_ln


def kernel(**inputs):
    N = inputs["x"].shape[0]
    NP = int(math.ceil(N / N_CORES / 128.0)) * 128
    in_maps, fast_ln = _prep(inputs, NP)

    key = (NP, fast_ln)
    if key not in _PROGRAM_CACHE:
        _PROGRAM_CACHE[key] = _build(NP, fast_ln)
    nc = _PROGRAM_CACHE[key]

    res = run_bass_kernel_spmd(nc, in_maps, list(range(N_CORES)))
    out = np.concatenate([res.results[c]["y"] for c in range(N_CORES)], axis=0)
    return out[:N]


if __name__ == "__main__":
    # quick self-test with small N
    rng = np.random.default_rng(0)
    Nt = 40960
    inputs = dict(
        x=rng.standard_normal((Nt, DIM), dtype=np.float32),
        coord=rng.random((Nt, 3), dtype=np.float32),
        patch_ids=rng.integers(0, P, size=(Nt,), dtype=np.int32),
        W1=rng.standard_normal((3, HID), dtype=np.float32) * 0.5,
        b1=rng.standard_normal((HID,), dtype=np.float32) * 0.5,
        W2=rng.standard_normal((HID, DIM), dtype=np.float32) * 0.1,
        b2=rng.standard_normal((DIM,), dtype=np.float32) * 0.1,
        dw_w=rng.standard_normal((DIM,), dtype=np.float32),
        dw_b=rng.standard_normal((DIM,), dtype=np.float32),
        pw_W=rng.standard_normal((DIM, DIM), dtype=np.float32) * 0.12,
        pw_b=rng.standard_normal((DIM,), dtype=np.float32) * 0.12,
        ln_g=np.ones((DIM,), np.float32),
        ln_b=np.zeros((DIM,), np.float32),
    )
    got = kernel(**inputs)

    def ref(i):
        sums = np.zeros((P, 3))
        np.add.at(sums, i["patch_ids"], i["coord"].astype(np.float64))
        cnt = np.bincount(i["patch_ids"], minlength=P).astype(np.float64)
        centers = sums / np.maximum(cnt, 1.0)[:, None]
        delta = i["coord"] - centers[i["patch_ids"]]
        import scipy.special as sp

        h = delta @ i["W1"].astype(np.float64) + i["b1"]
        g = 0.5 * h * (1 + sp.erf(h / np.sqrt(2.0)))
        pos = g @ i["W2"].astype(np.float64) + i["b2"]
        out = i["x"] + pos
        yv = out * i["dw_w"] + i["dw_b"]
        out = yv @ i["pw_W"].astype(np.float64) + i["pw_b"]
        muv = out.mean(-1, keepdims=True)
        varv = ((out - muv) ** 2).mean(-1, keepdims=True)
        return (out - muv) / np.sqrt(varv + EPS) * i["ln_g"] + i["ln_b"]

    want = ref(inputs)
    err = np.abs(got - want) / (np.abs(want) + 1e-3)
    print("max rel err:", err.max(), "mean:", err.mean())



# revision 8
# speedup vs baseline: 350.8950x; 350.8950x over previous
"""Trainium2 Bass kernel for CPELite (gnn_message_passing).

Data-parallel over N=1M points on 8 NeuronCores. Per core:
  phase H: partial segment-sum of (coord, 1) over 4096 patches via
           bf16 one-hot radix (hi=128 x lo=32) matmuls into PSUM.
  AllReduce of the [4096,4] table; centers = sums/max(cnt,1) kept in
  SBUF as a radix-split table (bf16 main + bf16 residual).
  phase M: per 2048-pt supertile: build transposed hi one-hot via
           gpsimd partition-broadcast + compare, gather centers with
           per-tile PE matmuls (radix hi via matmul, lo via masked
           reduce), delta = coord-center, PE transpose to
           feature-major, h = gelu(W1^T deltaT + b1),
           out2T = W2p^T h + Wp^T xT + bpp, transpose back, LayerNorm.

Weight folding (host): Wp = diag(dw_w) @ pw_W ; W2p = W2 @ Wp ;
 bpp = b2 @ Wp + dw_b @ pw_W + pw_b. So out2 = ((x+pos)*dw_w+dw_b) @ pw_W
 + pw_b == x @ Wp + gelu(...) @ W2p + bpp.

The indirect-DMA gather of the previous version (128 x 16B descriptors
per 128-pt tile) cost ~2.3 ms/core on hardware; the PE-based gather
replaces it with dense matmuls.
"""

import math

import ml_dtypes
import numpy as np

import concourse.bacc as bacc
import concourse.bass as bass
import concourse.mybir as mybir
import concourse.tile as tile
from concourse.bass_utils import run_bass_kernel_spmd
from concourse.masks import make_identity

N_CORES = 8
DIM = 64
HID = 64
P = 4096
EPS = 1e-5

F32 = mybir.dt.float32
BF16 = mybir.dt.bfloat16

_PROGRAM_CACHE = {}


def _build(NP, fast_ln, repeat=1, collective=True):
    """Build the per-core SPMD program. NP = padded points per core.

    repeat>1 re-runs the full pipeline (phase H + AllReduce + phase M)
    for wall-clock slope timing in test.py. collective=False (sim-only)
    replaces the AllReduce with a local copy."""
    assert NP % 128 == 0
    n_tiles = NP // 128
    ST = 16  # point-tiles per supertile
    HT = 8  # point-tiles per PSUM half-tile
    n_full = n_tiles // ST
    tail = n_tiles - n_full * ST
    st_sizes = [ST] * n_full + ([tail] if tail else [])

    nc = bacc.Bacc("TRN2", num_swdge_queues=4)

    x_in = nc.dram_tensor("x", [NP, DIM], F32, kind="ExternalInput")
    coord_in = nc.dram_tensor("coord", [NP, 3], F32, kind="ExternalInput")
    pidhi_in = nc.dram_tensor("pidhi", [NP], BF16, kind="ExternalInput")
    pidlo_in = nc.dram_tensor("pidlo", [NP], BF16, kind="ExternalInput")
    w1_in = nc.dram_tensor("w1", [3, HID], F32, kind="ExternalInput")
    b1_in = nc.dram_tensor("b1", [HID, 1], F32, kind="ExternalInput")
    w2p_in = nc.dram_tensor("w2p", [HID, DIM], F32, kind="ExternalInput")
    wp_in = nc.dram_tensor("wp", [DIM, DIM], F32, kind="ExternalInput")
    bpp_in = nc.dram_tensor("bpp", [DIM, 1], F32, kind="ExternalInput")
    chi_in = nc.dram_tensor("chi", [128, 128], BF16, kind="ExternalInput")
    clo_in = nc.dram_tensor("clo", [128, 32], BF16, kind="ExternalInput")
    chicol_in = nc.dram_tensor("chicol", [128, 1], BF16, kind="ExternalInput")
    grep_in = nc.dram_tensor("grep", [128, DIM], F32, kind="ExternalInput")
    brep_in = nc.dram_tensor("brep", [128, DIM], F32, kind="ExternalInput")
    y_out = nc.dram_tensor("y", [NP, DIM], F32, kind="ExternalOutput")

    with tile.TileContext(nc) as tc:
        with (
            tc.tile_pool(name="const", bufs=1) as cpool,
            tc.tile_pool(name="dram", bufs=1, space="DRAM") as dpool,
        ):
            # constants
            w1pad = cpool.tile([67, HID], F32)
            nc.sync.dma_start(out=w1pad[64:67, :], in_=w1_in[:])
            w1_sb = w1pad[64:67, :]
            b1_sb = cpool.tile([HID, 1], F32)
            nc.sync.dma_start(out=b1_sb[:], in_=b1_in[:])
            w2p_sb = cpool.tile([HID, DIM], F32)
            nc.sync.dma_start(out=w2p_sb[:], in_=w2p_in[:])
            wp_sb = cpool.tile([DIM, DIM], F32)
            nc.sync.dma_start(out=wp_sb[:], in_=wp_in[:])
            bpp_sb = cpool.tile([DIM, 1], F32)
            nc.sync.dma_start(out=bpp_sb[:], in_=bpp_in[:])
            chi_sb = cpool.tile([128, 128], BF16)
            nc.sync.dma_start(out=chi_sb[:], in_=chi_in[:])
            clo_sb = cpool.tile([128, 32], BF16)
            nc.sync.dma_start(out=clo_sb[:], in_=clo_in[:])
            chicol_sb = cpool.tile([128, 1], BF16)
            nc.sync.dma_start(out=chicol_sb[:], in_=chicol_in[:])
            ident_sb = cpool.tile([128, 128], F32)
            make_identity(nc, ident_sb[:])
            eps_sb = cpool.tile([128, 1], F32)
            nc.vector.memset(eps_sb[:], EPS)
            if not fast_ln:
                grep_sb = cpool.tile([128, DIM], F32)
                nc.sync.dma_start(out=grep_sb[:], in_=grep_in[:])
                brep_sb = cpool.tile([128, DIM], F32)
                nc.sync.dma_start(out=brep_sb[:], in_=brep_in[:])
            # centers radix tables, rebuilt per repetition after AllReduce
            cbf_sb = cpool.tile([128, 3, 32], BF16)
            cres_sb = cpool.tile([128, 3, 32], BF16)

            hist_dram = dpool.tile([P, 4], F32)
            hist_ar = dpool.tile([P, 4], F32)

            for _rep in range(repeat):
                # ---------------- phase H: histogram ----------------
                with (
                    tc.tile_pool(name="hsb", bufs=3) as hpool,
                    tc.tile_pool(name="hwork", bufs=2) as wpool,
                    tc.tile_pool(name="hps", bufs=1, space="PSUM") as hps,
                ):
                    psum_hist = hps.tile([128, 128], F32, space="PSUM")
                    gtile = 0
                    t0 = 0
                    for nt in st_sizes:
                        npts = nt * 128
                        phi = hpool.tile([128, ST], BF16, tag="phi")
                        nc.sync.dma_start(
                            out=phi[:, :nt],
                            in_=pidhi_in[t0 : t0 + npts].rearrange(
                                "(j p) -> p j", p=128
                            ),
                        )
                        plo = hpool.tile([128, ST], BF16, tag="plo")
                        nc.sync.dma_start(
                            out=plo[:, :nt],
                            in_=pidlo_in[t0 : t0 + npts].rearrange(
                                "(j p) -> p j", p=128
                            ),
                        )
                        c4 = hpool.tile([128, ST, 4], F32, tag="c4")
                        nc.vector.memset(c4[:, :nt, 3], 1.0)
                        nc.sync.dma_start(
                            out=c4[:, :nt, 0:3],
                            in_=coord_in[t0 : t0 + npts, :].rearrange(
                                "(j p) c -> p j c", p=128
                            ),
                        )
                        oh_hi = wpool.tile([128, ST, 128], BF16, tag="ohhi")
                        nc.vector.tensor_tensor(
                            out=oh_hi[:, :nt],
                            in0=phi[:, :nt, None].to_broadcast([128, nt, 128]),
                            in1=chi_sb[:, None, :].to_broadcast([128, nt, 128]),
                            op=mybir.AluOpType.is_equal,
                        )
                        oh_lo = wpool.tile([128, ST, 32], BF16, tag="ohlo")
                        nc.vector.tensor_tensor(
                            out=oh_lo[:, :nt],
                            in0=plo[:, :nt, None].to_broadcast([128, nt, 32]),
                            in1=clo_sb[:, None, :].to_broadcast([128, nt, 32]),
                            op=mybir.AluOpType.is_equal,
                        )
                        r_all = wpool.tile([128, ST, 4, 32], BF16, tag="rall")
                        nc.vector.tensor_tensor(
                            out=r_all[:, :nt],
                            in0=c4[:, :nt, :, None].to_broadcast([128, nt, 4, 32]),
                            in1=oh_lo[:, :nt, None, :].to_broadcast([128, nt, 4, 32]),
                            op=mybir.AluOpType.mult,
                        )
                        for j in range(nt):
                            nc.tensor.matmul(
                                out=psum_hist[:],
                                lhsT=r_all[:, j].rearrange("p a b -> p (a b)"),
                                rhs=oh_hi[:, j],
                                start=(gtile == 0),
                                stop=(gtile == n_tiles - 1),
                            )
                            gtile += 1
                        t0 += npts
                    hist_sb = wpool.tile([128, 128], F32, tag="hsb")
                    nc.vector.tensor_copy(out=hist_sb[:], in_=psum_hist[:])
                    # hist_sb[(d,lo), hi] -> hist_dram[(hi,lo), d]
                    for d in range(4):
                        nc.sync.dma_start(
                            out=hist_dram[:]
                            .rearrange("(h l) d -> l h d", l=32)[:, :, d]
                            .rearrange("l h -> l h"),
                            in_=hist_sb[32 * d : 32 * d + 32, :],
                        )

                # all-reduce + centers radix tables
                with tc.tile_pool(name="cwork", bufs=1) as cw:
                    if collective:
                        nc.gpsimd.collective_compute(
                            "AllReduce",
                            mybir.AluOpType.add,
                            replica_groups=[list(range(N_CORES))],
                            ins=[hist_dram.opt()],
                            outs=[hist_ar.opt()],
                        )
                    else:
                        nc.sync.dma_start(out=hist_ar[:], in_=hist_dram[:])
                    htab = cw.tile([128, 32, 4], F32)
                    nc.sync.dma_start(
                        out=htab[:],
                        in_=hist_ar[:].rearrange("(h l) d -> h (l d)", l=32),
                    )
                    cnt = cw.tile([128, 32], F32)
                    nc.vector.tensor_scalar(
                        out=cnt[:],
                        in0=htab[:, :, 3],
                        scalar1=1.0,
                        scalar2=None,
                        op0=mybir.AluOpType.max,
                    )
                    rec = cw.tile([128, 32], F32)
                    nc.vector.reciprocal(out=rec[:], in_=cnt[:])
                    ctab = cw.tile([128, 32, 4], F32)
                    nc.vector.tensor_tensor(
                        out=ctab[:],
                        in0=htab[:],
                        in1=rec[:, :, None].to_broadcast([128, 32, 4]),
                        op=mybir.AluOpType.mult,
                    )
                    # [hi, lo, d] -> [hi, d, lo], split into bf16 + residual
                    ctabT = cw.tile([128, 3, 32], F32)
                    for d in range(3):
                        nc.vector.tensor_copy(
                            out=ctabT[:, d, :], in_=ctab[:, :, d]
                        )
                    nc.vector.tensor_copy(out=cbf_sb[:], in_=ctabT[:])
                    nc.vector.tensor_tensor(
                        out=cres_sb[:],
                        in0=ctabT[:],
                        in1=cbf_sb[:],
                        op=mybir.AluOpType.subtract,
                    )

                # ---------------- phase M: main ----------------
                with (
                    tc.tile_pool(name="min", bufs=3) as mpool,
                    tc.tile_pool(name="mmid", bufs=2) as mid,
                    tc.tile_pool(name="mout", bufs=2) as mo,
                    tc.tile_pool(name="mps", bufs=2, space="PSUM") as mps,
                    tc.tile_pool(name="gps", bufs=2, space="PSUM") as gps,
                ):
                    t0 = 0
                    for sti, nt in enumerate(st_sizes):
                        npts = nt * 128
                        W = nt * 128  # wide free dim

                        xc = mpool.tile([128, ST, 68], F32, tag="xc")
                        nc.sync.dma_start(
                            out=xc[:, :nt, 0:64],
                            in_=x_in[t0 : t0 + npts, :].rearrange(
                                "(j p) f -> p j f", p=128
                            ),
                        )
                        nc.sync.dma_start(
                            out=xc[:, :nt, 64:67],
                            in_=coord_in[t0 : t0 + npts, :].rearrange(
                                "(j p) c -> p j c", p=128
                            ),
                        )
                        # --- centers gather: hi via PE matmul, lo via mask ---
                        pidlo = mpool.tile([128, ST], BF16, tag="pidlo")
                        nc.sync.dma_start(
                            out=pidlo[:, :nt],
                            in_=pidlo_in[t0 : t0 + npts].rearrange(
                                "(j p) -> p j", p=128
                            ),
                        )
                        phrow = mpool.tile([1, ST * 128], BF16, tag="phrow")
                        nc.sync.dma_start(
                            out=phrow[0:1, :npts],
                            in_=pidhi_in[t0 : t0 + npts].rearrange(
                                "(o n) -> o n", o=1
                            ),
                        )
                        bchi = mid.tile([128, ST * 128], BF16, tag="bchi")
                        nc.gpsimd.partition_broadcast(
                            out_ap=bchi[:, :W], in_ap=phrow[:, :W]
                        )
                        ohT = mid.tile([128, ST * 128], BF16, tag="ohT")
                        nc.vector.tensor_tensor(
                            out=ohT[:, :W],
                            in0=bchi[:, :W],
                            in1=chicol_sb[:, 0:1].to_broadcast([128, W]),
                            op=mybir.AluOpType.is_equal,
                        )
                        ohlo = mid.tile([128, ST, 32], BF16, tag="ohlo")
                        nc.vector.tensor_tensor(
                            out=ohlo[:, :nt],
                            in0=pidlo[:, :nt, None].to_broadcast([128, nt, 32]),
                            in1=clo_sb[:, None, :].to_broadcast([128, nt, 32]),
                            op=mybir.AluOpType.is_equal,
                        )
                        for h0 in range(0, nt, HT):
                            hn = min(HT, nt - h0)
                            ps_g = gps.tile([128, HT, 128], F32, space="PSUM", tag="g")
                            for jj in range(hn):
                                j = h0 + jj
                                lhsT = ohT[:, j * 128 : (j + 1) * 128]
                                nc.tensor.matmul(
                                    out=ps_g[:, jj, 0:96],
                                    lhsT=lhsT,
                                    rhs=cbf_sb[:].rearrange("p a b -> p (a b)"),
                                    start=True,
                                    stop=False,
                                )
                                nc.tensor.matmul(
                                    out=ps_g[:, jj, 0:96],
                                    lhsT=lhsT,
                                    rhs=cres_sb[:].rearrange("p a b -> p (a b)"),
                                    start=False,
                                    stop=True,
                                )
                            scr = mid.tile([128, HT, 3, 32], F32, tag="scr")
                            nc.vector.tensor_tensor(
                                out=scr[:, :hn],
                                in0=ps_g[:, :hn, 0:96].rearrange(
                                    "p j (d l) -> p j d l", d=3
                                ),
                                in1=ohlo[:, h0 : h0 + hn, None, :].to_broadcast(
                                    [128, hn, 3, 32]
                                ),
                                op=mybir.AluOpType.mult,
                            )
                            cg = mid.tile([128, HT, 3], F32, tag="cg")
                            nc.vector.tensor_reduce(
                                out=cg[:, :hn],
                                in_=scr[:, :hn],
                                axis=mybir.AxisListType.X,
                                op=mybir.AluOpType.add,
                            )
                            nc.vector.tensor_tensor(
                                out=xc[:, h0 : h0 + hn, 64:67],
                                in0=xc[:, h0 : h0 + hn, 64:67],
                                in1=cg[:, :hn],
                                op=mybir.AluOpType.subtract,
                            )
                        # --- transpose in, mlp, transpose out, LN ---
                        xcT = mid.tile([68, ST * 128], F32, tag="xcT")
                        g_sb = mid.tile([HID, ST * 128], F32, tag="gsb")
                        out2 = mid.tile([DIM, ST * 128], F32, tag="out2")
                        y_sb = mo.tile([128, ST, DIM], F32, tag="ysb")
                        for h0 in range(0, nt, HT):
                            hn = min(HT, nt - h0)
                            HW = hn * 128
                            c0 = h0 * 128
                            ps_in = mps.tile([68, HT * 128], F32, space="PSUM", tag="ps")
                            for jj in range(hn):
                                nc.tensor.transpose(
                                    out=ps_in[:, jj * 128 : (jj + 1) * 128],
                                    in_=xc[:, h0 + jj, 0:68],
                                    identity=ident_sb[:],
                                )
                            nc.vector.tensor_copy(
                                out=xcT[:, c0 : c0 + HW], in_=ps_in[:, :HW]
                            )
                            # mlp1
                            ps_h = mps.tile([HID, HT * 128], F32, space="PSUM", tag="ps")
                            for q in range(0, HW, 512):
                                qe = min(q + 512, HW)
                                nc.tensor.matmul(
                                    out=ps_h[:, q:qe],
                                    lhsT=w1_sb,
                                    rhs=xcT[64:67, c0 + q : c0 + qe],
                                    start=True,
                                    stop=True,
                                )
                            nc.scalar.activation(
                                out=g_sb[:, c0 : c0 + HW],
                                in_=ps_h[:, :HW],
                                func=mybir.ActivationFunctionType.Gelu,
                                bias=b1_sb[:],
                                scale=1.0,
                            )
                            # mlp2 + pw
                            ps_o = mps.tile([DIM, HT * 128], F32, space="PSUM", tag="ps")
                            for q in range(0, HW, 512):
                                qe = min(q + 512, HW)
                                nc.tensor.matmul(
                                    out=ps_o[:, q:qe],
                                    lhsT=w2p_sb[:],
                                    rhs=g_sb[:, c0 + q : c0 + qe],
                                    start=True,
                                    stop=False,
                                )
                            for q in range(0, HW, 512):
                                qe = min(q + 512, HW)
                                nc.tensor.matmul(
                                    out=ps_o[:, q:qe],
                                    lhsT=wp_sb[:],
                                    rhs=xcT[0:64, c0 + q : c0 + qe],
                                    start=False,
                                    stop=True,
                                )
                            # copy + bias (bpp as per-partition column)
                            nc.vector.tensor_tensor(
                                out=out2[:, c0 : c0 + HW],
                                in0=ps_o[:, :HW],
                                in1=bpp_sb[:, 0:1].to_broadcast([DIM, HW]),
                                op=mybir.AluOpType.add,
                            )
                            # transpose out
                            ps_y = mps.tile([128, HT, DIM], F32, space="PSUM", tag="ps")
                            for jj in range(hn):
                                nc.tensor.transpose(
                                    out=ps_y[:, jj, :],
                                    in_=out2[:, (h0 + jj) * 128 : (h0 + jj + 1) * 128],
                                    identity=ident_sb[0:64, 0:64],
                                )
                            # LayerNorm on [128, hn, 64]
                            w_view = ps_y[:, :hn, :]
                            mu = mo.tile([128, HT], F32, tag="mu")
                            nc.vector.tensor_reduce(
                                out=mu[:, :hn],
                                in_=w_view,
                                axis=mybir.AxisListType.X,
                                op=mybir.AluOpType.add,
                            )
                            nc.vector.tensor_scalar(
                                out=mu[:, :hn],
                                in0=mu[:, :hn],
                                scalar1=1.0 / DIM,
                                scalar2=None,
                                op0=mybir.AluOpType.mult,
                            )
                            scr2 = mo.tile([128, HT, DIM], F32, tag="scr2")
                            nc.scalar.activation(
                                out=scr2[:, :hn],
                                in_=w_view,
                                func=mybir.ActivationFunctionType.Square,
                            )
                            ssq = mo.tile([128, HT], F32, tag="ssq")
                            nc.vector.tensor_reduce(
                                out=ssq[:, :hn],
                                in_=scr2[:, :hn],
                                axis=mybir.AxisListType.X,
                                op=mybir.AluOpType.add,
                            )
                            var = mo.tile([128, HT], F32, tag="var")
                            nc.vector.tensor_scalar(
                                out=ssq[:, :hn],
                                in0=ssq[:, :hn],
                                scalar1=1.0 / DIM,
                                scalar2=None,
                                op0=mybir.AluOpType.mult,
                            )
                            nc.vector.tensor_tensor(
                                out=var[:, :hn],
                                in0=mu[:, :hn],
                                in1=mu[:, :hn],
                                op=mybir.AluOpType.mult,
                            )
                            nc.vector.tensor_tensor(
                                out=var[:, :hn],
                                in0=ssq[:, :hn],
                                in1=var[:, :hn],
                                op=mybir.AluOpType.subtract,
                            )
                            sd = mo.tile([128, HT], F32, tag="sd")
                            nc.scalar.activation(
                                out=sd[:, :hn],
                                in_=var[:, :hn],
                                func=mybir.ActivationFunctionType.Sqrt,
                                bias=eps_sb[:],
                            )
                            inv = mo.tile([128, HT], F32, tag="inv")
                            nc.vector.reciprocal(out=inv[:, :hn], in_=sd[:, :hn])
                            t1 = mo.tile([128, HT, DIM], F32, tag="t1")
                            nc.vector.tensor_tensor(
                                out=t1[:, :hn],
                                in0=w_view,
                                in1=mu[:, :hn, None].to_broadcast([128, hn, DIM]),
                                op=mybir.AluOpType.subtract,
                            )
                            nc.vector.tensor_tensor(
                                out=y_sb[:, h0 : h0 + hn],
                                in0=t1[:, :hn],
                                in1=inv[:, :hn, None].to_broadcast([128, hn, DIM]),
                                op=mybir.AluOpType.mult,
                            )
                            if not fast_ln:
                                nc.vector.tensor_tensor(
                                    out=y_sb[:, h0 : h0 + hn],
                                    in0=y_sb[:, h0 : h0 + hn],
                                    in1=grep_sb[:, None, :].to_broadcast(
                                        [128, hn, DIM]
                                    ),
                                    op=mybir.AluOpType.mult,
                                )
                                nc.vector.tensor_tensor(
                                    out=y_sb[:, h0 : h0 + hn],
                                    in0=y_sb[:, h0 : h0 + hn],
                                    in1=brep_sb[:, None, :].to_broadcast(
                                        [128, hn, DIM]
                                    ),
                                    op=mybir.AluOpType.add,
                                )
                        nc.sync.dma_start(
                            out=y_out[t0 : t0 + npts, :].rearrange(
                                "(j p) f -> p j f", p=128
                            ),
                            in_=y_sb[:, :nt],
                        )
                        t0 += npts

    nc.finalize()
    return nc


def _prep(inputs, NP):
    """Host-side prep: fold weights, build per-core input maps."""
    x = np.ascontiguousarray(np.asarray(inputs["x"], dtype=np.float32))
    coord = np.ascontiguousarray(np.asarray(inputs["coord"], dtype=np.float32))
    pid = np.asarray(inputs["patch_ids"]).astype(np.int64)
    N = x.shape[0]

    W1 = np.asarray(inputs["W1"], dtype=np.float64)
    b1 = np.asarray(inputs["b1"], dtype=np.float64)
    W2 = np.asarray(inputs["W2"], dtype=np.float64)
    b2 = np.asarray(inputs["b2"], dtype=np.float64)
    dw_w = np.asarray(inputs["dw_w"], dtype=np.float64)
    dw_b = np.asarray(inputs["dw_b"], dtype=np.float64)
    pw_W = np.asarray(inputs["pw_W"], dtype=np.float64)
    pw_b = np.asarray(inputs["pw_b"], dtype=np.float64)
    ln_g = np.asarray(inputs["ln_g"], dtype=np.float64)
    ln_b = np.asarray(inputs["ln_b"], dtype=np.float64)

    Wp = dw_w[:, None] * pw_W
    W2p = W2 @ Wp
    bpp = b2 @ Wp + dw_b @ pw_W + pw_b

    fast_ln = bool(np.all(ln_g == 1.0) and np.all(ln_b == 0.0))

    chi = np.broadcast_to(np.arange(128, dtype=np.float32), (128, 128))
    clo = np.broadcast_to(np.arange(32, dtype=np.float32), (128, 32))
    chicol = np.arange(128, dtype=np.float32).reshape(128, 1)

    common = dict(
        w1=np.asarray(W1, dtype=np.float32),
        b1=np.asarray(b1, dtype=np.float32).reshape(HID, 1),
        w2p=np.asarray(W2p, dtype=np.float32),
        wp=np.asarray(Wp, dtype=np.float32),
        bpp=np.asarray(bpp, dtype=np.float32).reshape(DIM, 1),
        chi=chi.astype(ml_dtypes.bfloat16),
        clo=clo.astype(ml_dtypes.bfloat16),
        chicol=chicol.astype(ml_dtypes.bfloat16),
        grep=np.broadcast_to(ln_g.astype(np.float32), (128, DIM)).copy(),
        brep=np.broadcast_to(ln_b.astype(np.float32), (128, DIM)).copy(),
    )

    pid_hi = (pid >> 5).astype(np.float32)
    pid_lo = (pid & 31).astype(np.float32)

    in_maps = []
    for c in range(N_CORES):
        a, b = c * NP, min((c + 1) * NP, N)
        pad = NP - (b - a)
        if pad == 0:
            xs, cs = x[a:b], coord[a:b]
            hi_s, lo_s = pid_hi[a:b], pid_lo[a:b]
        else:
            xs = np.concatenate([x[a:b], np.zeros((pad, DIM), np.float32)])
            cs = np.concatenate([coord[a:b], np.zeros((pad, 3), np.float32)])
            hi_s = np.concatenate([pid_hi[a:b], np.full(pad, 1000.0, np.float32)])
            lo_s = np.concatenate([pid_lo[a:b], np.full(pad, 1000.0, np.float32)])
        m = dict(common)
        m.update(
            x=xs,
            coord=cs,
            pidhi=hi_s.astype(ml_dtypes.bfloat16),
            pidlo=lo_s.astype(ml_dtypes.bfloat16),
        )
        in_maps.append(m)
    return in_maps, fast_ln


def kernel(**inputs):
    N = inputs["x"].shape[0]
    NP = int(math.ceil(N / N_CORES / 128.0)) * 128
    in_maps, fast_ln = _prep(inputs, NP)

    key = (NP, fast_ln)
    if key not in _PROGRAM_CACHE:
        _PROGRAM_CACHE[key] = _build(NP, fast_ln)
    nc = _PROGRAM_CACHE[key]

    res = run_bass_kernel_spmd(nc, in_maps, list(range(N_CORES)))
    out = np.concatenate([res.results[c]["y"] for c in range(N_CORES)], axis=0)
    return out[:N]


if __name__ == "__main__":
    # quick self-test with small N
    rng = np.random.default_rng(0)
    Nt = 40960
    inputs = dict(
        x=rng.standard_normal((Nt, DIM), dtype=np.float32),
        coord=rng.random((Nt, 3), dtype=np.float32),
        patch_ids=rng.integers(0, P, size=(Nt,), dtype=np.int32),
        W1=rng.standard_normal((3, HID), dtype=np.float32) * 0.5,
        b1=rng.standard_normal((HID,), dtype=np.float32) * 0.5,
        W2=rng.standard_normal((HID, DIM), dtype=np.float32) * 0.1,
        b2=rng.standard_normal((DIM,), dtype=np.float32) * 0.1,
        dw_w=rng.standard_normal((DIM,), dtype=np.float32),
        dw_b=rng.standard_normal((DIM,), dtype=np.float32),
        pw_W=rng.standard_normal((DIM, DIM), dtype=np.float32) * 0.12,
        pw_b=rng.standard_normal((DIM,), dtype=np.float32) * 0.12,
        ln_g=np.ones((DIM,), np.float32),
        ln_b=np.zeros((DIM,), np.float32),
    )
    got = kernel(**inputs)

    def ref(i):
        sums = np.zeros((P, 3))
        np.add.at(sums, i["patch_ids"], i["coord"].astype(np.float64))
        cnt = np.bincount(i["patch_ids"], minlength=P).astype(np.float64)
        centers = sums / np.maximum(cnt, 1.0)[:, None]
        delta = i["coord"] - centers[i["patch_ids"]]
        import scipy.special as sp

        h = delta @ i["W1"].astype(np.float64) + i["b1"]
        g = 0.5 * h * (1 + sp.erf(h / np.sqrt(2.0)))
        pos = g @ i["W2"].astype(np.float64) + i["b2"]
        out = i["x"] + pos
        yv = out * i["dw_w"] + i["dw_b"]
        out = yv @ i["pw_W"].astype(np.float64) + i["pw_b"]
        muv = out.mean(-1, keepdims=True)
        varv = ((out - muv) ** 2).mean(-1, keepdims=True)
        return (out - muv) / np.sqrt(varv + EPS) * i["ln_g"] + i["ln_b"]

    want = ref(inputs)
    err = np.abs(got - want) / (np.abs(want) + 1e-3)
    print("max rel err:", err.max(), "mean:", err.mean())


# revision 9
# speedup vs baseline: 561.0766x; 1.5990x over previous
"""Trainium2 Bass kernel for CPELite (gnn_message_passing).

Data-parallel over N=1M points on 8 NeuronCores. Per core:
  phase H: partial segment-sum of (coord, 1) over 4096 patches via
           bf16 one-hot radix (hi=128 x lo=32) matmuls into PSUM.
  AllReduce of the [4096,4] table; centers = sums/max(cnt,1) kept in
  SBUF as a radix-split table (bf16 main + bf16 residual).
  phase M: per 2048-pt supertile: build transposed hi one-hot via
           gpsimd partition-broadcast + compare, gather centers with
           per-tile PE matmuls (radix hi via matmul, lo via masked
           reduce), delta = coord-center, PE transpose to
           feature-major, h = gelu(W1^T deltaT + b1),
           out2T = W2p^T h + Wp^T xT + bpp, transpose back, LayerNorm.

Weight folding (host): Wp = diag(dw_w) @ pw_W ; W2p = W2 @ Wp ;
 bpp = b2 @ Wp + dw_b @ pw_W + pw_b. So out2 = ((x+pos)*dw_w+dw_b) @ pw_W
 + pw_b == x @ Wp + gelu(...) @ W2p + bpp.

The indirect-DMA gather of the previous version (128 x 16B descriptors
per 128-pt tile) cost ~2.3 ms/core on hardware; the PE-based gather
replaces it with dense matmuls.
"""

import math

import ml_dtypes
import numpy as np

import concourse.bacc as bacc
import concourse.bass as bass
import concourse.mybir as mybir
import concourse.tile as tile
from concourse.bass_utils import run_bass_kernel_spmd
from concourse.masks import make_identity

N_CORES = 8
DIM = 64
HID = 64
P = 4096
EPS = 1e-5

F32 = mybir.dt.float32
BF16 = mybir.dt.bfloat16

_PROGRAM_CACHE = {}


def _build(NP, fast_ln, repeat=1, collective=True):
    """Build the per-core SPMD program. NP = padded points per core.

    repeat>1 re-runs the full pipeline (phase H + AllReduce + phase M)
    for wall-clock slope timing in test.py. collective=False (sim-only)
    replaces the AllReduce with a local copy."""
    assert NP % 128 == 0
    n_tiles = NP // 128
    ST = 16  # point-tiles per supertile
    HT = 8  # point-tiles per PSUM half-tile
    n_full = n_tiles // ST
    tail = n_tiles - n_full * ST
    st_sizes = [ST] * n_full + ([tail] if tail else [])

    nc = bacc.Bacc("TRN2", num_swdge_queues=4)

    x_in = nc.dram_tensor("x", [NP, DIM], F32, kind="ExternalInput")
    coord_in = nc.dram_tensor("coord", [NP, 3], F32, kind="ExternalInput")
    pidhi_in = nc.dram_tensor("pidhi", [NP], BF16, kind="ExternalInput")
    pidlo_in = nc.dram_tensor("pidlo", [NP], BF16, kind="ExternalInput")
    w1_in = nc.dram_tensor("w1", [3, HID], F32, kind="ExternalInput")
    b1_in = nc.dram_tensor("b1", [HID, 1], F32, kind="ExternalInput")
    w2p_in = nc.dram_tensor("w2p", [HID, DIM], F32, kind="ExternalInput")
    wp_in = nc.dram_tensor("wp", [DIM, DIM], F32, kind="ExternalInput")
    bpp_in = nc.dram_tensor("bpp", [DIM, 1], F32, kind="ExternalInput")
    chi_in = nc.dram_tensor("chi", [128, 128], BF16, kind="ExternalInput")
    clo_in = nc.dram_tensor("clo", [128, 32], BF16, kind="ExternalInput")
    chicol_in = nc.dram_tensor("chicol", [128, 1], BF16, kind="ExternalInput")
    grep_in = nc.dram_tensor("grep", [128, DIM], F32, kind="ExternalInput")
    brep_in = nc.dram_tensor("brep", [128, DIM], F32, kind="ExternalInput")
    y_out = nc.dram_tensor("y", [NP, DIM], F32, kind="ExternalOutput")

    with tile.TileContext(nc) as tc:
        with (
            tc.tile_pool(name="const", bufs=1) as cpool,
            tc.tile_pool(name="dram", bufs=1, space="DRAM") as dpool,
        ):
            # constants
            w1pad = cpool.tile([67, HID], F32)
            nc.sync.dma_start(out=w1pad[64:67, :], in_=w1_in[:])
            w1_sb = w1pad[64:67, :]
            b1_sb = cpool.tile([HID, 1], F32)
            nc.sync.dma_start(out=b1_sb[:], in_=b1_in[:])
            w2p_sb = cpool.tile([HID, DIM], F32)
            nc.sync.dma_start(out=w2p_sb[:], in_=w2p_in[:])
            wp_sb = cpool.tile([DIM, DIM], F32)
            nc.sync.dma_start(out=wp_sb[:], in_=wp_in[:])
            bpp_sb = cpool.tile([DIM, 1], F32)
            nc.sync.dma_start(out=bpp_sb[:], in_=bpp_in[:])
            chi_sb = cpool.tile([128, 128], BF16)
            nc.sync.dma_start(out=chi_sb[:], in_=chi_in[:])
            clo_sb = cpool.tile([128, 32], BF16)
            nc.sync.dma_start(out=clo_sb[:], in_=clo_in[:])
            chicol_sb = cpool.tile([128, 1], BF16)
            nc.sync.dma_start(out=chicol_sb[:], in_=chicol_in[:])
            ident_sb = cpool.tile([128, 128], F32)
            make_identity(nc, ident_sb[:])
            eps_sb = cpool.tile([128, 1], F32)
            nc.vector.memset(eps_sb[:], EPS)
            if not fast_ln:
                grep_sb = cpool.tile([128, DIM], F32)
                nc.sync.dma_start(out=grep_sb[:], in_=grep_in[:])
                brep_sb = cpool.tile([128, DIM], F32)
                nc.sync.dma_start(out=brep_sb[:], in_=brep_in[:])
            # centers radix tables, rebuilt per repetition after AllReduce
            cbf_sb = cpool.tile([128, 3, 32], BF16)
            cres_sb = cpool.tile([128, 3, 32], BF16)

            hist_dram = dpool.tile([P, 4], F32)
            hist_ar = dpool.tile([P, 4], F32)

            for _rep in range(repeat):
                # ---------------- phase H: histogram ----------------
                with (
                    tc.tile_pool(name="hsb", bufs=3) as hpool,
                    tc.tile_pool(name="hwork", bufs=2) as wpool,
                    tc.tile_pool(name="hps", bufs=1, space="PSUM") as hps,
                ):
                    psum_hist = hps.tile([128, 128], F32, space="PSUM")
                    gtile = 0
                    t0 = 0
                    for nt in st_sizes:
                        npts = nt * 128
                        phi = hpool.tile([128, ST], BF16, tag="phi")
                        nc.sync.dma_start(
                            out=phi[:, :nt],
                            in_=pidhi_in[t0 : t0 + npts].rearrange(
                                "(j p) -> p j", p=128
                            ),
                        )
                        plo = hpool.tile([128, ST], BF16, tag="plo")
                        nc.sync.dma_start(
                            out=plo[:, :nt],
                            in_=pidlo_in[t0 : t0 + npts].rearrange(
                                "(j p) -> p j", p=128
                            ),
                        )
                        c4 = hpool.tile([128, ST, 4], F32, tag="c4")
                        nc.vector.memset(c4[:, :nt, 3], 1.0)
                        nc.sync.dma_start(
                            out=c4[:, :nt, 0:3],
                            in_=coord_in[t0 : t0 + npts, :].rearrange(
                                "(j p) c -> p j c", p=128
                            ),
                        )
                        oh_hi = wpool.tile([128, ST, 128], BF16, tag="ohhi")
                        nc.vector.tensor_tensor(
                            out=oh_hi[:, :nt],
                            in0=phi[:, :nt, None].to_broadcast([128, nt, 128]),
                            in1=chi_sb[:, None, :].to_broadcast([128, nt, 128]),
                            op=mybir.AluOpType.is_equal,
                        )
                        oh_lo = wpool.tile([128, ST, 32], BF16, tag="ohlo")
                        nc.vector.tensor_tensor(
                            out=oh_lo[:, :nt],
                            in0=plo[:, :nt, None].to_broadcast([128, nt, 32]),
                            in1=clo_sb[:, None, :].to_broadcast([128, nt, 32]),
                            op=mybir.AluOpType.is_equal,
                        )
                        r_all = wpool.tile([128, ST, 4, 32], BF16, tag="rall")
                        nc.vector.tensor_tensor(
                            out=r_all[:, :nt],
                            in0=c4[:, :nt, :, None].to_broadcast([128, nt, 4, 32]),
                            in1=oh_lo[:, :nt, None, :].to_broadcast([128, nt, 4, 32]),
                            op=mybir.AluOpType.mult,
                        )
                        for j in range(nt):
                            nc.tensor.matmul(
                                out=psum_hist[:],
                                lhsT=r_all[:, j].rearrange("p a b -> p (a b)"),
                                rhs=oh_hi[:, j],
                                start=(gtile == 0),
                                stop=(gtile == n_tiles - 1),
                            )
                            gtile += 1
                        t0 += npts
                    hist_sb = wpool.tile([128, 128], F32, tag="hsb")
                    nc.vector.tensor_copy(out=hist_sb[:], in_=psum_hist[:])
                    # hist_sb[(d,lo), hi] -> hist_dram[(hi,lo), d]
                    for d in range(4):
                        nc.sync.dma_start(
                            out=hist_dram[:]
                            .rearrange("(h l) d -> l h d", l=32)[:, :, d]
                            .rearrange("l h -> l h"),
                            in_=hist_sb[32 * d : 32 * d + 32, :],
                        )

                # all-reduce + centers radix tables
                with tc.tile_pool(name="cwork", bufs=1) as cw:
                    if collective:
                        nc.gpsimd.collective_compute(
                            "AllReduce",
                            mybir.AluOpType.add,
                            replica_groups=[list(range(N_CORES))],
                            ins=[hist_dram.opt()],
                            outs=[hist_ar.opt()],
                        )
                    else:
                        nc.sync.dma_start(out=hist_ar[:], in_=hist_dram[:])
                    htab = cw.tile([128, 32, 4], F32)
                    nc.sync.dma_start(
                        out=htab[:],
                        in_=hist_ar[:].rearrange("(h l) d -> h (l d)", l=32),
                    )
                    cnt = cw.tile([128, 32], F32)
                    nc.vector.tensor_scalar(
                        out=cnt[:],
                        in0=htab[:, :, 3],
                        scalar1=1.0,
                        scalar2=None,
                        op0=mybir.AluOpType.max,
                    )
                    rec = cw.tile([128, 32], F32)
                    nc.vector.reciprocal(out=rec[:], in_=cnt[:])
                    ctab = cw.tile([128, 32, 4], F32)
                    nc.vector.tensor_tensor(
                        out=ctab[:],
                        in0=htab[:],
                        in1=rec[:, :, None].to_broadcast([128, 32, 4]),
                        op=mybir.AluOpType.mult,
                    )
                    # [hi, lo, d] -> [hi, d, lo], split into bf16 + residual
                    ctabT = cw.tile([128, 3, 32], F32)
                    for d in range(3):
                        nc.vector.tensor_copy(
                            out=ctabT[:, d, :], in_=ctab[:, :, d]
                        )
                    nc.vector.tensor_copy(out=cbf_sb[:], in_=ctabT[:])
                    nc.vector.tensor_tensor(
                        out=cres_sb[:],
                        in0=ctabT[:],
                        in1=cbf_sb[:],
                        op=mybir.AluOpType.subtract,
                    )

                # ---------------- phase M: main ----------------
                with (
                    tc.tile_pool(name="min", bufs=4) as mpool,
                    tc.tile_pool(name="mmid", bufs=3) as mid,
                    tc.tile_pool(name="mout", bufs=2) as mo,
                    tc.tile_pool(name="mps", bufs=2, space="PSUM") as mps,
                    tc.tile_pool(name="gps", bufs=2, space="PSUM") as gps,
                ):
                    t0 = 0
                    for sti, nt in enumerate(st_sizes):
                        npts = nt * 128
                        W = nt * 128  # wide free dim

                        xc = mpool.tile([128, ST, 68], F32, tag="xc")
                        nc.sync.dma_start(
                            out=xc[:, :nt, 0:64],
                            in_=x_in[t0 : t0 + npts, :].rearrange(
                                "(j p) f -> p j f", p=128
                            ),
                        )
                        nc.sync.dma_start(
                            out=xc[:, :nt, 64:67],
                            in_=coord_in[t0 : t0 + npts, :].rearrange(
                                "(j p) c -> p j c", p=128
                            ),
                        )
                        # --- centers gather: hi via PE matmul, lo via mask ---
                        pidlo = mpool.tile([128, ST], BF16, tag="pidlo")
                        nc.sync.dma_start(
                            out=pidlo[:, :nt],
                            in_=pidlo_in[t0 : t0 + npts].rearrange(
                                "(j p) -> p j", p=128
                            ),
                        )
                        phrow = mpool.tile([1, ST * 128], BF16, tag="phrow")
                        nc.sync.dma_start(
                            out=phrow[0:1, :npts],
                            in_=pidhi_in[t0 : t0 + npts].rearrange(
                                "(o n) -> o n", o=1
                            ),
                        )
                        bchi = mid.tile([128, ST * 128], BF16, tag="bchi")
                        nc.gpsimd.partition_broadcast(
                            out_ap=bchi[:, :W], in_ap=phrow[:, :W]
                        )
                        ohT = mid.tile([128, ST * 128], BF16, tag="ohT")
                        nc.vector.tensor_tensor(
                            out=ohT[:, :W],
                            in0=bchi[:, :W],
                            in1=chicol_sb[:, 0:1].to_broadcast([128, W]),
                            op=mybir.AluOpType.is_equal,
                        )
                        ohlo = mid.tile([128, ST, 32], BF16, tag="ohlo")
                        nc.vector.tensor_tensor(
                            out=ohlo[:, :nt],
                            in0=pidlo[:, :nt, None].to_broadcast([128, nt, 32]),
                            in1=clo_sb[:, None, :].to_broadcast([128, nt, 32]),
                            op=mybir.AluOpType.is_equal,
                        )
                        for h0 in range(0, nt, HT):
                            hn = min(HT, nt - h0)
                            ps_g = gps.tile([128, HT, 128], F32, space="PSUM", tag="g")
                            for jj in range(hn):
                                j = h0 + jj
                                lhsT = ohT[:, j * 128 : (j + 1) * 128]
                                nc.tensor.matmul(
                                    out=ps_g[:, jj, 0:96],
                                    lhsT=lhsT,
                                    rhs=cbf_sb[:].rearrange("p a b -> p (a b)"),
                                    start=True,
                                    stop=False,
                                )
                                nc.tensor.matmul(
                                    out=ps_g[:, jj, 0:96],
                                    lhsT=lhsT,
                                    rhs=cres_sb[:].rearrange("p a b -> p (a b)"),
                                    start=False,
                                    stop=True,
                                )
                            scr = mid.tile([128, HT, 3, 32], F32, tag="scr")
                            nc.vector.tensor_tensor(
                                out=scr[:, :hn],
                                in0=ps_g[:, :hn, 0:96].rearrange(
                                    "p j (d l) -> p j d l", d=3
                                ),
                                in1=ohlo[:, h0 : h0 + hn, None, :].to_broadcast(
                                    [128, hn, 3, 32]
                                ),
                                op=mybir.AluOpType.mult,
                            )
                            cg = mid.tile([128, HT, 3], F32, tag="cg")
                            nc.vector.tensor_reduce(
                                out=cg[:, :hn],
                                in_=scr[:, :hn],
                                axis=mybir.AxisListType.X,
                                op=mybir.AluOpType.add,
                            )
                            nc.vector.tensor_tensor(
                                out=xc[:, h0 : h0 + hn, 64:67],
                                in0=xc[:, h0 : h0 + hn, 64:67],
                                in1=cg[:, :hn],
                                op=mybir.AluOpType.subtract,
                            )
                        # --- transpose in, mlp, transpose out, LN ---
                        xcT = mid.tile([68, ST * 128], F32, tag="xcT")
                        g_sb = mid.tile([HID, ST * 128], F32, tag="gsb")
                        out2 = mid.tile([DIM, ST * 128], F32, tag="out2")
                        y_sb = mo.tile([128, ST, DIM], F32, tag="ysb")
                        for h0 in range(0, nt, HT):
                            hn = min(HT, nt - h0)
                            HW = hn * 128
                            c0 = h0 * 128
                            ps_in = mps.tile([68, HT * 128], F32, space="PSUM", tag="ps")
                            for jj in range(hn):
                                nc.tensor.transpose(
                                    out=ps_in[:, jj * 128 : (jj + 1) * 128],
                                    in_=xc[:, h0 + jj, 0:68],
                                    identity=ident_sb[:],
                                )
                            nc.vector.tensor_copy(
                                out=xcT[:, c0 : c0 + HW], in_=ps_in[:, :HW]
                            )
                            # mlp1
                            ps_h = mps.tile([HID, HT * 128], F32, space="PSUM", tag="ps")
                            for q in range(0, HW, 512):
                                qe = min(q + 512, HW)
                                nc.tensor.matmul(
                                    out=ps_h[:, q:qe],
                                    lhsT=w1_sb,
                                    rhs=xcT[64:67, c0 + q : c0 + qe],
                                    start=True,
                                    stop=True,
                                )
                            nc.scalar.activation(
                                out=g_sb[:, c0 : c0 + HW],
                                in_=ps_h[:, :HW],
                                func=mybir.ActivationFunctionType.Gelu,
                                bias=b1_sb[:],
                                scale=1.0,
                            )
                            # mlp2 + pw
                            ps_o = mps.tile([DIM, HT * 128], F32, space="PSUM", tag="ps")
                            for q in range(0, HW, 512):
                                qe = min(q + 512, HW)
                                nc.tensor.matmul(
                                    out=ps_o[:, q:qe],
                                    lhsT=w2p_sb[:],
                                    rhs=g_sb[:, c0 + q : c0 + qe],
                                    start=True,
                                    stop=False,
                                )
                            for q in range(0, HW, 512):
                                qe = min(q + 512, HW)
                                nc.tensor.matmul(
                                    out=ps_o[:, q:qe],
                                    lhsT=wp_sb[:],
                                    rhs=xcT[0:64, c0 + q : c0 + qe],
                                    start=False,
                                    stop=True,
                                )
                            # copy + bias (bpp as per-partition column)
                            nc.vector.tensor_tensor(
                                out=out2[:, c0 : c0 + HW],
                                in0=ps_o[:, :HW],
                                in1=bpp_sb[:, 0:1].to_broadcast([DIM, HW]),
                                op=mybir.AluOpType.add,
                            )
                            # transpose out
                            ps_y = mps.tile([128, HT, DIM], F32, space="PSUM", tag="ps")
                            for jj in range(hn):
                                nc.tensor.transpose(
                                    out=ps_y[:, jj, :],
                                    in_=out2[:, (h0 + jj) * 128 : (h0 + jj + 1) * 128],
                                    identity=ident_sb[0:64, 0:64],
                                )
                            # LayerNorm on [128, hn, 64]
                            w_view = ps_y[:, :hn, :]
                            mu = mo.tile([128, HT], F32, tag="mu")
                            nc.vector.tensor_reduce(
                                out=mu[:, :hn],
                                in_=w_view,
                                axis=mybir.AxisListType.X,
                                op=mybir.AluOpType.add,
                            )
                            nc.vector.tensor_scalar(
                                out=mu[:, :hn],
                                in0=mu[:, :hn],
                                scalar1=1.0 / DIM,
                                scalar2=None,
                                op0=mybir.AluOpType.mult,
                            )
                            scr2 = mo.tile([128, HT, DIM], F32, tag="scr2")
                            nc.scalar.activation(
                                out=scr2[:, :hn],
                                in_=w_view,
                                func=mybir.ActivationFunctionType.Square,
                            )
                            ssq = mo.tile([128, HT], F32, tag="ssq")
                            nc.vector.tensor_reduce(
                                out=ssq[:, :hn],
                                in_=scr2[:, :hn],
                                axis=mybir.AxisListType.X,
                                op=mybir.AluOpType.add,
                            )
                            var = mo.tile([128, HT], F32, tag="var")
                            nc.vector.tensor_scalar(
                                out=ssq[:, :hn],
                                in0=ssq[:, :hn],
                                scalar1=1.0 / DIM,
                                scalar2=None,
                                op0=mybir.AluOpType.mult,
                            )
                            nc.vector.tensor_tensor(
                                out=var[:, :hn],
                                in0=mu[:, :hn],
                                in1=mu[:, :hn],
                                op=mybir.AluOpType.mult,
                            )
                            nc.vector.tensor_tensor(
                                out=var[:, :hn],
                                in0=ssq[:, :hn],
                                in1=var[:, :hn],
                                op=mybir.AluOpType.subtract,
                            )
                            sd = mo.tile([128, HT], F32, tag="sd")
                            nc.scalar.activation(
                                out=sd[:, :hn],
                                in_=var[:, :hn],
                                func=mybir.ActivationFunctionType.Sqrt,
                                bias=eps_sb[:],
                            )
                            inv = mo.tile([128, HT], F32, tag="inv")
                            nc.vector.reciprocal(out=inv[:, :hn], in_=sd[:, :hn])
                            t1 = mo.tile([128, HT, DIM], F32, tag="t1")
                            nc.vector.tensor_tensor(
                                out=t1[:, :hn],
                                in0=w_view,
                                in1=mu[:, :hn, None].to_broadcast([128, hn, DIM]),
                                op=mybir.AluOpType.subtract,
                            )
                            nc.vector.tensor_tensor(
                                out=y_sb[:, h0 : h0 + hn],
                                in0=t1[:, :hn],
                                in1=inv[:, :hn, None].to_broadcast([128, hn, DIM]),
                                op=mybir.AluOpType.mult,
                            )
                            if not fast_ln:
                                nc.vector.tensor_tensor(
                                    out=y_sb[:, h0 : h0 + hn],
                                    in0=y_sb[:, h0 : h0 + hn],
                                    in1=grep_sb[:, None, :].to_broadcast(
                                        [128, hn, DIM]
                                    ),
                                    op=mybir.AluOpType.mult,
                                )
                                nc.vector.tensor_tensor(
                                    out=y_sb[:, h0 : h0 + hn],
                                    in0=y_sb[:, h0 : h0 + hn],
                                    in1=brep_sb[:, None, :].to_broadcast(
                                        [128, hn, DIM]
                                    ),
                                    op=mybir.AluOpType.add,
                                )
                        nc.sync.dma_start(
                            out=y_out[t0 : t0 + npts, :].rearrange(
                                "(j p) f -> p j f", p=128
                            ),
                            in_=y_sb[:, :nt],
                        )
                        t0 += npts

    nc.finalize()
    return nc


def _prep(inputs, NP):
    """Host-side prep: fold weights, build per-core input maps."""
    x = np.ascontiguousarray(np.asarray(inputs["x"], dtype=np.float32))
    coord = np.ascontiguousarray(np.asarray(inputs["coord"], dtype=np.float32))
    pid = np.asarray(inputs["patch_ids"]).astype(np.int64)
    N = x.shape[0]

    W1 = np.asarray(inputs["W1"], dtype=np.float64)
    b1 = np.asarray(inputs["b1"], dtype=np.float64)
    W2 = np.asarray(inputs["W2"], dtype=np.float64)
    b2 = np.asarray(inputs["b2"], dtype=np.float64)
    dw_w = np.asarray(inputs["dw_w"], dtype=np.float64)
    dw_b = np.asarray(inputs["dw_b"], dtype=np.float64)
    pw_W = np.asarray(inputs["pw_W"], dtype=np.float64)
    pw_b = np.asarray(inputs["pw_b"], dtype=np.float64)
    ln_g = np.asarray(inputs["ln_g"], dtype=np.float64)
    ln_b = np.asarray(inputs["ln_b"], dtype=np.float64)

    Wp = dw_w[:, None] * pw_W
    W2p = W2 @ Wp
    bpp = b2 @ Wp + dw_b @ pw_W + pw_b

    fast_ln = bool(np.all(ln_g == 1.0) and np.all(ln_b == 0.0))

    chi = np.broadcast_to(np.arange(128, dtype=np.float32), (128, 128))
    clo = np.broadcast_to(np.arange(32, dtype=np.float32), (128, 32))
    chicol = np.arange(128, dtype=np.float32).reshape(128, 1)

    common = dict(
        w1=np.asarray(W1, dtype=np.float32),
        b1=np.asarray(b1, dtype=np.float32).reshape(HID, 1),
        w2p=np.asarray(W2p, dtype=np.float32),
        wp=np.asarray(Wp, dtype=np.float32),
        bpp=np.asarray(bpp, dtype=np.float32).reshape(DIM, 1),
        chi=chi.astype(ml_dtypes.bfloat16),
        clo=clo.astype(ml_dtypes.bfloat16),
        chicol=chicol.astype(ml_dtypes.bfloat16),
        grep=np.broadcast_to(ln_g.astype(np.float32), (128, DIM)).copy(),
        brep=np.broadcast_to(ln_b.astype(np.float32), (128, DIM)).copy(),
    )

    pid_hi = (pid >> 5).astype(np.float32)
    pid_lo = (pid & 31).astype(np.float32)

    in_maps = []
    for c in range(N_CORES):
        a, b = c * NP, min((c + 1) * NP, N)
        pad = NP - (b - a)
        if pad == 0:
            xs, cs = x[a:b], coord[a:b]
            hi_s, lo_s = pid_hi[a:b], pid_lo[a:b]
        else:
            xs = np.concatenate([x[a:b], np.zeros((pad, DIM), np.float32)])
            cs = np.concatenate([coord[a:b], np.zeros((pad, 3), np.float32)])
            hi_s = np.concatenate([pid_hi[a:b], np.full(pad, 1000.0, np.float32)])
            lo_s = np.concatenate([pid_lo[a:b], np.full(pad, 1000.0, np.float32)])
        m = dict(common)
        m.update(
            x=xs,
            coord=cs,
            pidhi=hi_s.astype(ml_dtypes.bfloat16),
            pidlo=lo_s.astype(ml_dtypes.bfloat16),
        )
        in_maps.append(m)
    return in_maps, fast_ln


def kernel(**inputs):
    N = inputs["x"].shape[0]
    NP = int(math.ceil(N / N_CORES / 128.0)) * 128
    in_maps, fast_ln = _prep(inputs, NP)

    key = (NP, fast_ln)
    if key not in _PROGRAM_CACHE:
        _PROGRAM_CACHE[key] = _build(NP, fast_ln)
    nc = _PROGRAM_CACHE[key]

    res = run_bass_kernel_spmd(nc, in_maps, list(range(N_CORES)))
    out = np.concatenate([res.results[c]["y"] for c in range(N_CORES)], axis=0)
    return out[:N]


if __name__ == "__main__":
    # quick self-test with small N
    rng = np.random.default_rng(0)
    Nt = 40960
    inputs = dict(
        x=rng.standard_normal((Nt, DIM), dtype=np.float32),
        coord=rng.random((Nt, 3), dtype=np.float32),
        patch_ids=rng.integers(0, P, size=(Nt,), dtype=np.int32),
        W1=rng.standard_normal((3, HID), dtype=np.float32) * 0.5,
        b1=rng.standard_normal((HID,), dtype=np.float32) * 0.5,
        W2=rng.standard_normal((HID, DIM), dtype=np.float32) * 0.1,
        b2=rng.standard_normal((DIM,), dtype=np.float32) * 0.1,
        dw_w=rng.standard_normal((DIM,), dtype=np.float32),
        dw_b=rng.standard_normal((DIM,), dtype=np.float32),
        pw_W=rng.standard_normal((DIM, DIM), dtype=np.float32) * 0.12,
        pw_b=rng.standard_normal((DIM,), dtype=np.float32) * 0.12,
        ln_g=np.ones((DIM,), np.float32),
        ln_b=np.zeros((DIM,), np.float32),
    )
    got = kernel(**inputs)

    def ref(i):
        sums = np.zeros((P, 3))
        np.add.at(sums, i["patch_ids"], i["coord"].astype(np.float64))
        cnt = np.bincount(i["patch_ids"], minlength=P).astype(np.float64)
        centers = sums / np.maximum(cnt, 1.0)[:, None]
        delta = i["coord"] - centers[i["patch_ids"]]
        import scipy.special as sp

        h = delta @ i["W1"].astype(np.float64) + i["b1"]
        g = 0.5 * h * (1 + sp.erf(h / np.sqrt(2.0)))
        pos = g @ i["W2"].astype(np.float64) + i["b2"]
        out = i["x"] + pos
        yv = out * i["dw_w"] + i["dw_b"]
        out = yv @ i["pw_W"].astype(np.float64) + i["pw_b"]
        muv = out.mean(-1, keepdims=True)
        varv = ((out - muv) ** 2).mean(-1, keepdims=True)
        return (out - muv) / np.sqrt(varv + EPS) * i["ln_g"] + i["ln_b"]

    want = ref(inputs)
    err = np.abs(got - want) / (np.abs(want) + 1e-3)
    print("max rel err:", err.max(), "mean:", err.mean())
